# revision 1
# baseline (speedup 1.0000x reference)
"""Attention2D Trainium2 Bass kernel.

Reference computation (per batch image, C=512 channels, N=1024 tokens):
    qkv = qkv_w @ x + qkv_b            # (1536, N)
    q,k,v per head (8 heads, head_dim 64)
    attn = softmax(scale * q.T k)      # (N, N) per head, scale = C**-0.5
    out  = v @ attn.T                  # (64, N) per head
    y    = x + proj_w @ out + proj_b

Sharding: data-parallel over batch. 16 images / 8 cores = 2 images per core.
Weights are replicated; no collectives.

Layout strategy (no transposes needed anywhere):
  - x kept as [C, N] (channels on partitions).
  - Q, K computed as [c_head, n] (lhsT = W^T chunk, rhs = x chunk).
  - V computed directly transposed: V^T [n, c] (lhsT = x chunk, rhs = W_v^T),
    stored in 65-wide per-head groups with a ones column at offset 64.
  - S^T[m, n] = matmul(lhsT=K[64, m-chunk], rhs=Q[64, n]) per head (K=64
    contraction; two heads land on PE row-groups 0-1 / 2-3 via base partition).
  - expS^T = Exp(SCALE * S^T) on the scalar engine, psum -> sbuf.
  - O~[c, n] = sum_m V^T'[m, c+ones] expS^T[m, n]: matmul with lhsT = V^T'
    [m-chunk, 65], accumulated over 8 m-chunks into psum [65, N]. Row 64 is
    the softmax denominator (courtesy of the ones column) -- zero extra cost.
  - normalize: DMA-broadcast row 64 across 64 partitions, DVE reciprocal+mul.
  - proj: lhsT = proj_w^T chunks, rhs = normalized O [c, n]; residual added
    from host-precomputed xr = x + proj_b.
"""

import os

import numpy as np
import ml_dtypes

import concourse.bass as bass
import concourse.tile as tile
from concourse import mybir
from concourse.bass_utils import run_bass_kernel_spmd

B, C, N = 16, 512, 1024
HEADS, HD = 8, 64
SCALE = float(C) ** -0.5
NCORES = 8
BPC = B // NCORES  # images per core

# matmul operand mode: "f32" (exact, 4 cyc/col), "f32r" (fp32 data, fast PE
# path), "bf16" (operands rounded to bf16, fp32 accumulation)
MM_MODE = os.environ.get("ATTN_MM_MODE", "f32r")

F32 = mybir.dt.float32


def _split_multi_waits(nc):
    """Walrus codegen in this toolchain rejects instructions carrying more
    than one semaphore wait ("Too many sync wait commands"). Hoist all but
    the last wait of such instructions into standalone InstEventSemaphore
    ops just before them (same engine, so per-engine order is preserved)."""
    n_split = 0
    for f in nc.m.functions:
        for b in f.blocks:
            out = []
            changed = False
            for inst in b.instructions:
                si = inst.sync_info
                waits = list(si.on_wait) if si is not None else []
                if len(waits) > 1:
                    for k, w in enumerate(waits[:-1]):
                        wi = mybir.InstEventSemaphore(
                            name=f"{inst.name}-presync{k}", ins=[], outs=[],
                            sync_info=mybir.SyncInfo(on_wait=[w], on_update=[]),
                        )
                        wi.engine = inst.engine
                        out.append(wi)
                        n_split += 1
                    inst.sync_info = mybir.SyncInfo(
                        on_wait=[waits[-1]], on_update=list(si.on_update)
                    )
                    changed = True
                out.append(inst)
            if changed:
                b.instructions = out
    return n_split


def _mdt(mode):
    if mode == "bf16":
        return mybir.dt.bfloat16
    if mode == "f32r":
        return mybir.dt.float32r
    return mybir.dt.float32


def _np_mdt(mode):
    return ml_dtypes.bfloat16 if mode == "bf16" else np.float32


def build_nc(mode=MM_MODE):
    mdt = _mdt(mode)

    def mm(ap):
        return ap

    nc = bass.Bass()
    xm_h = nc.dram_tensor("xm", [BPC, C, N], mdt, kind="ExternalInput")
    xr_h = nc.dram_tensor("xr", [BPC, C, N], F32, kind="ExternalInput")
    wqkv_h = nc.dram_tensor("wqkv", [C, 3 * C], mdt, kind="ExternalInput")
    pw_h = nc.dram_tensor("pw", [C, C], mdt, kind="ExternalInput")
    bqk_h = nc.dram_tensor("bqk", [128, 8], F32, kind="ExternalInput")
    bv_h = nc.dram_tensor("bv", [128, C], F32, kind="ExternalInput")
    y_h = nc.dram_tensor("y", [BPC, C, N], F32, kind="ExternalOutput")

    CC = C // 128          # 4 contraction chunks of x channels
    NH = N // 512          # moving-dim halves
    MC = N // 128          # m-chunks (key/value token chunks)
    dma = nc.sync.dma_start

    # sbuf pool buffer counts (per-partition bytes are the scarce resource)
    OCP = False
    BUFS = dict(
        xm=6, xr=3, qk=10, vt=13,
        es=7, on=4, rbc=2, stg=2, y=2, ocp=2,
    )

    with tile.TileContext(nc) as tc:
        with (
            tc.tile_pool(name="w", bufs=1) as wp,
            tc.tile_pool(name="sb", bufs=2) as sb,
            tc.tile_pool(name="ps", bufs=2, space=bass.MemorySpace.PSUM) as ps,
            tc.tile_pool(name="pso", bufs=2, space=bass.MemorySpace.PSUM) as pso,
            tc.tile_pool(name="dr", bufs=4, space=bass.MemorySpace.DRAM) as dr,
        ):
            def load_weights():
                bqk = wp.tile([128, 8], F32, tag="bqk", name="bqk")
                nc.gpsimd.dma_start(out=bqk[:], in_=bqk_h[:])
                bv = wp.tile([128, C], F32, tag="bv", name="bv")
                nc.gpsimd.dma_start(out=bv[:], in_=bv_h[:])
                for cc in range(CC):
                    t = wp.tile([128, C], mdt, tag=f"pw{cc}", name=f"pw{cc}")
                    nc.gpsimd.dma_start(out=t[:], in_=pw_h[cc * 128:(cc + 1) * 128, :])
                    pw_sb.append(t)
                return bqk, bv

            wqkv_sb, pw_sb = [], []
            xm_sb = {}   # (img, cc) -> tile
            xr_sb = {}   # (img, oc) -> tile
            qk_sb = {}   # (img, oc) -> tile
            vt_sb = {}   # (img, mc) -> tile
            on_sb = {}   # (img, cc) -> tile
            es_tiles = {}

            def load_xm(img):
                for cc in range(CC):
                    t = sb.tile([128, N], mdt, tag="xm", bufs=BUFS["xm"],
                                name=f"xm{img}_{cc}")
                    for nh in range(NH):
                        dma(out=t[:, nh * 512:(nh + 1) * 512],
                            in_=xm_h[img, cc * 128:(cc + 1) * 128,
                                     nh * 512:(nh + 1) * 512])
                    xm_sb[(img, cc)] = t

            def load_xr(img):
                for oc in range(CC):
                    t = sb.tile([128, N], F32, tag="xr", bufs=BUFS["xr"],
                                name=f"xr{img}_{oc}")
                    nc.gpsimd.dma_start(out=t[:], in_=xr_h[img, oc * 128:(oc + 1) * 128, :])
                    xr_sb[(img, oc)] = t

            def emit_qkv(img, ocs):
                for oc in ocs:
                    q_ps = ps.tile([128, N], F32, tag="s")
                    for nh in range(NH):
                        for cc in range(CC):
                            nc.tensor.matmul(
                                q_ps[:, nh * 512:(nh + 1) * 512],
                                mm(wqkv_sb[cc][:, oc * 128:(oc + 1) * 128]),
                                mm(xm_sb[(img, cc)][:, nh * 512:(nh + 1) * 512]),
                                start=(cc == 0), stop=(cc == CC - 1),
                            )
                    t = sb.tile([128, N], mdt, tag="qk", bufs=BUFS["qk"],
                                name=f"qk{img}_{oc}")
                    nc.vector.tensor_scalar_add(t[:], q_ps[:], bqk_sb[:, oc:oc + 1])
                    qk_sb[(img, oc)] = t

            def emit_v(img, mcs):
                for mc in mcs:
                    v_ps = ps.tile([128, 512], F32, tag="s")
                    for cc in range(CC):
                        nc.tensor.matmul(
                            v_ps[:],
                            mm(xm_sb[(img, cc)][:, mc * 128:(mc + 1) * 128]),
                            mm(wqkv_sb[cc][:, 2 * C:3 * C]),
                            start=(cc == 0), stop=(cc == CC - 1),
                        )
                    t = sb.tile([128, HEADS * 65], mdt, tag="vt", bufs=BUFS["vt"],
                                name=f"vt{img}_{mc}")
                    tv = t[:].rearrange("p (h u) -> p h u", u=65)
                    ones_view = tv[:, :, 64:65]
                    if mode == "f32r":  # memset can't write f32r directly
                        ones_view = ones_view.bitcast(F32)
                    nc.vector.memset(ones_view, 1.0)
                    nc.vector.tensor_add(
                        tv[:, :, 0:64],
                        v_ps[:].rearrange("p (h u) -> p h u", u=64),
                        bv_sb[:].rearrange("p (h u) -> p h u", u=64),
                    )
                    vt_sb[(img, mc)] = t

            def alloc_on(img):
                for i in range(CC):
                    on_sb[(img, i)] = sb.tile(
                        [128, N], mdt, tag="on", bufs=BUFS["on"],
                        name=f"on{img}_{i}")

            def emit_head(img, h, filler=None):
                # S^T -> exp -> O pipelined at m-chunk granularity (1-chunk
                # skew): only ~2-3 expS tiles are ever live, and the PE gets
                # S(mc+1) to chew on while ACT finishes exp(mc).
                pair, half = h // 2, h % 2
                base = 64 * half
                qt, kt = qk_sb[(img, pair)], qk_sb[(img, 4 + pair)]
                o_ps = pso.tile([65, N], F32, tag="o")
                es = {}

                def s_step(mc):
                    s_ps = ps.tile([128, N], F32, tag="s")
                    for nh in range(NH):
                        nc.tensor.matmul(
                            s_ps[:, nh * 512:(nh + 1) * 512],
                            mm(kt[base:base + 64, mc * 128:(mc + 1) * 128]),
                            mm(qt[base:base + 64, nh * 512:(nh + 1) * 512]),
                            start=True, stop=True,
                        )
                    e = sb.tile([128, N], mdt, tag="es", bufs=BUFS["es"])
                    nc.scalar.activation(
                        e[:], s_ps[:], mybir.ActivationFunctionType.Exp,
                        scale=SCALE,
                    )
                    es[mc] = e

                def o_step(mc):
                    e = es.pop(mc)
                    for nh in range(NH):
                        nc.tensor.matmul(
                            o_ps[:, nh * 512:(nh + 1) * 512],
                            mm(vt_sb[(img, mc)][:, h * 65:h * 65 + 65]),
                            mm(e[:, nh * 512:(nh + 1) * 512]),
                            start=(mc == 0), stop=(mc == MC - 1),
                            skip_group_check=True,
                        )

                SK = 3  # S->O skew depth: absorbs ACT's per-chunk exp lag
                for mc in range(MC):
                    s_step(mc)
                    if mc == 1 and filler is not None:
                        filler()
                    if mc >= SK:
                        o_step(mc - SK)
                for mc in range(MC - SK, MC):
                    o_step(mc)

                # normalize: psum row 64 holds the softmax denominator.
                # Copy psum -> sbuf first so the psum slot frees after one DVE
                # op instead of being held through the whole chain.
                if OCP:
                    ocp = sb.tile([65, N], F32, tag="ocp", bufs=BUFS["ocp"])
                    nc.vector.tensor_copy(ocp[:], o_ps[:])
                else:
                    ocp = o_ps
                rbc = sb.tile([65, N], F32, tag="rbc", bufs=BUFS["rbc"])
                nc.vector.reciprocal(rbc[64:65, :], ocp[64:65, :])
                rd = dr.tile([1, N], F32, tag="rd")
                dma(out=rd[:], in_=rbc[64:65, :])
                dma(out=rbc[0:64, :], in_=rd[:].partition_broadcast(64))
                if half == 0:
                    nc.vector.tensor_mul(
                        on_sb[(img, pair)][0:64, :], ocp[0:64, :], rbc[0:64, :])
                else:
                    stg = sb.tile([64, N], mdt, tag="stg", bufs=BUFS["stg"])
                    nc.vector.tensor_mul(stg[:], ocp[0:64, :], rbc[0:64, :])
                    nc.gpsimd.dma_start(out=on_sb[(img, pair)][64:128, :], in_=stg[:])

            def emit_proj(img, ocs):
                for oc in ocs:
                    p_ps = ps.tile([128, N], F32, tag="s")
                    for nh in range(NH):
                        for cc in range(CC):
                            nc.tensor.matmul(
                                p_ps[:, nh * 512:(nh + 1) * 512],
                                mm(pw_sb[cc][:, oc * 128:(oc + 1) * 128]),
                                mm(on_sb[(img, cc)][:, nh * 512:(nh + 1) * 512]),
                                start=(cc == 0), stop=(cc == CC - 1),
                            )
                    yt = sb.tile([128, N], F32, tag="y", bufs=BUFS["y"])
                    nc.vector.tensor_add(yt[:], p_ps[:], xr_sb[(img, oc)][:])
                    nc.gpsimd.dma_start(out=y_h[img, oc * 128:(oc + 1) * 128, :], in_=yt[:])

            # ---------- emission schedule (2 images, pipelined) ----
            # interleave x and weight DMAs in consumption order so the first
            # qkv matmuls unblock after one transfer per queue
            # warm the ACT exp table during the input DMAs
            warm = wp.tile([1, 1], F32, tag="warm", name="warm")
            nc.vector.memset(warm[:], 0.0)
            nc.scalar.activation(
                warm[:], warm[:], mybir.ActivationFunctionType.Exp)
            for cc in range(CC):
                t = sb.tile([128, N], mdt, tag="xm", bufs=BUFS["xm"],
                            name=f"xm0_{cc}")
                xm_sb[(0, cc)] = t
                w = wp.tile([128, 3 * C], mdt, tag=f"wqkv{cc}", name=f"wqkv{cc}")
                wqkv_sb.append(w)
                weng = nc.scalar if cc % 2 == 0 else nc.gpsimd
                xeng = nc.sync if cc % 2 == 0 else nc.scalar
                for nh in range(NH):
                    xeng.dma_start(
                        out=t[:, nh * 512:(nh + 1) * 512],
                        in_=xm_h[0, cc * 128:(cc + 1) * 128,
                                 nh * 512:(nh + 1) * 512])
                    weng.dma_start(
                        out=w[:, nh * C // 2:(nh + 1) * C // 2],
                        in_=wqkv_h[cc * 128:(cc + 1) * 128,
                                   nh * C // 2:(nh + 1) * C // 2])
                nc.gpsimd.dma_start(
                    out=w[:, C:3 * C],
                    in_=wqkv_h[cc * 128:(cc + 1) * 128, C:3 * C])
            bqk_sb, bv_sb = load_weights()
            emit_qkv(0, range(8))
            emit_v(0, range(MC))
            load_xm(1)          # prefetch during image-0 attention
            load_xr(0)
            alloc_on(0)

            head_order = [1, 0, 3, 2, 5, 4, 7, 6]  # odd first: the last
            # normalize of each pair is the direct DVE write, keeping the slow
            # stg-DMA path off the critical edge into proj

            # The attention loop is ACT-bound per head (8.3us exp vs 6.8us of
            # PE matmuls), while qkv/V/proj are PE-only. Drain those as filler
            # units between heads so neither engine idles.
            for pos, h in enumerate(head_order):
                f = None
                if pos >= 1:
                    if pos < 6:
                        f = (lambda p=pos: emit_qkv(1, [p - 1]))
                    else:
                        f = (lambda p=pos: (emit_qkv(1, [p - 1]),
                                            emit_v(1, [2 * (p - 6), 2 * (p - 6) + 1])))
                emit_head(0, h, filler=f)
            emit_qkv(1, [7])
            emit_v(1, range(4, MC))
            load_xr(1)
            alloc_on(1)
            for pos, h in enumerate(head_order):
                f = (lambda p=pos: emit_proj(0, [p - 1])) if 1 <= pos <= CC else None
                emit_head(1, h, filler=f)
            emit_proj(1, range(CC))

    _split_multi_waits(nc)
    return nc


_CACHE = {}


def _get_nc(mode):
    if mode not in _CACHE:
        _CACHE[mode] = build_nc(mode)
    return _CACHE[mode]


def prepare_inputs(x, qkv_w, qkv_b, proj_w, proj_b, mode=MM_MODE):
    npmdt = _np_mdt(mode)
    x = np.asarray(x, np.float32).reshape(B, C, N)
    qkv_w = np.asarray(qkv_w, np.float32)
    qkv_b = np.asarray(qkv_b, np.float32)
    proj_w = np.asarray(proj_w, np.float32)
    proj_b = np.asarray(proj_b, np.float32)

    xm = np.ascontiguousarray(x.astype(npmdt))
    xr = np.ascontiguousarray(x + proj_b[None, :, None])
    wqkv = np.ascontiguousarray(qkv_w.T.astype(npmdt))
    pw = np.ascontiguousarray(proj_w.T.astype(npmdt))
    bqk = np.ascontiguousarray(qkv_b[:1024].reshape(8, 128).T)
    bv = np.ascontiguousarray(np.broadcast_to(qkv_b[2 * C:], (128, C)))

    in_maps = []
    for c in range(NCORES):
        sl = slice(c * BPC, (c + 1) * BPC)
        in_maps.append({
            "xm": xm[sl], "xr": xr[sl], "wqkv": wqkv, "pw": pw,
            "bqk": bqk, "bv": bv,
        })
    return in_maps


def run(x, qkv_w, qkv_b, proj_w, proj_b, mode=MM_MODE, **spmd_kwargs):
    nc = _get_nc(mode)
    in_maps = prepare_inputs(x, qkv_w, qkv_b, proj_w, proj_b, mode)
    res = run_bass_kernel_spmd(nc, in_maps, list(range(NCORES)), **spmd_kwargs)
    y = np.concatenate([np.asarray(res.results[c]["y"]) for c in range(NCORES)], axis=0)
    return res, y.reshape(B, C, 32, 32).astype(np.float32)


def kernel(x, qkv_w, qkv_b, proj_w, proj_b):
    _, y = run(x, qkv_w, qkv_b, proj_w, proj_b)
    return y



# revision 9
# speedup vs baseline: 1.1625x; 1.1625x over previous
"""Attention2D Trainium2 Bass kernel — fp8 DoubleRow edition.

Reference computation (per batch image, C=512 channels, N=1024 tokens):
    qkv = qkv_w @ x + qkv_b            # (1536, N)
    q,k,v per head (8 heads, head_dim 64)
    attn = softmax(scale * q.T k)      # (N, N) per head, scale = C**-0.5
    out  = v @ attn.T                  # (64, N) per head
    y    = x + proj_w @ out + proj_b

Sharding: data-parallel over batch. 16 images / 8 cores = 2 images per core.
Weights replicated; no collectives.

Numerics/performance strategy (validated offline: rel err ~2.6e-3 vs the
2e-2 gate):
  - Everything quantized to fp8 e4m3. All the big channel-contraction
    matmuls (qkv, V, attn@V, proj) run in MatmulPerfMode.DoubleRow: one
    instruction contracts 2 k-tiles of 128 at 0.5 cycles/col (4x f32r).
    Layout for DR: lhsT [K,2,M], rhs [K,2,N] with the k-tile pair packed
    in the free dim.
  - S = q.T k has contraction 64 only, stays fp8 non-DR (1 cyc/col) with
    the head pair packed on partition halves (base partition 0/64).
  - K bias is dropped entirely: softmax(q.(k~+bk)) == softmax(q.k~ + f(n));
    the per-n term cancels. Q keeps its bias (supplies the bq.k~ term).
  - V bias folded into the residual on the host: sum_m attn = 1, so
    out_att = Vnorm + bv and y gains the constant proj_w@bv, merged into
    xr = x + proj_b + proj_w@bv.
  - exp is the elementwise bottleneck (2*8*1024^2 elems/core); only ACT
    and DVE can read PSUM. Split: ACT runs exact Exp (also absorbs the
    Q/K/V psum->sbuf fp8 conversions; 'exp_and_others' table covers Exp+
    Copy+Identity so no table reloads); DVE runs a Schraudolph-style
    integer exp: i8 = (S*scale*8*log2e) + (56.5-0.35), bitcast int8 ->
    fp8e4m3 (bias 7, 3 mantissa bits). Rounding-mode miscalibration is
    common-mode and cancels through the softmax denominator.
  - Softmax denominator comes free from a ones-column in the V^T tiles
    (row 64 of the attn@V psum). reciprocal on DVE, partition-broadcast
    on GPSIMD (sbuf->sbuf, no DRAM round trip), normalize-mul on DVE.
"""

import math

import numpy as np
import ml_dtypes

import concourse.bass as bass
import concourse.tile as tile
from concourse import mybir
from concourse.bass_utils import run_bass_kernel_spmd

B, C, N = 16, 512, 1024
HEADS, HD = 8, 64
SCALE = float(C) ** -0.5
NCORES = 8
BPC = B // NCORES  # images per core

F32 = mybir.dt.float32
F8 = mybir.dt.float8e4
I8 = mybir.dt.int8

# Schraudolph exp -> fp8e4m3 constants: byte = round(x*SCALE*8*log2(e) + 56.5
# - 0.35). +56 = bias 7 << 3; +0.5 turns trunc into round; -0.35 recenters the
# piecewise-linear 2^frac error.
EXP_C1 = SCALE * 8.0 / math.log(2.0)
EXP_C2 = 56.5 - 0.35

DR = mybir.MatmulPerfMode.DoubleRow


def _split_multi_waits(nc):
    """Walrus codegen rejects instructions carrying more than one semaphore
    wait. Hoist all but the last wait into standalone InstEventSemaphore ops
    just before them (same engine, so per-engine order is preserved)."""
    n_split = 0
    for f in nc.m.functions:
        for b in f.blocks:
            out = []
            changed = False
            for inst in b.instructions:
                si = inst.sync_info
                waits = list(si.on_wait) if si is not None else []
                if len(waits) > 1:
                    for k, w in enumerate(waits[:-1]):
                        wi = mybir.InstEventSemaphore(
                            name=f"{inst.name}-presync{k}", ins=[], outs=[],
                            sync_info=mybir.SyncInfo(on_wait=[w], on_update=[]),
                        )
                        wi.engine = inst.engine
                        out.append(wi)
                        n_split += 1
                    inst.sync_info = mybir.SyncInfo(
                        on_wait=[waits[-1]], on_update=list(si.on_update)
                    )
                    changed = True
                out.append(inst)
            if changed:
                b.instructions = out
    return n_split


# Per-(phase, head-position) exp engine split: list of 8 engines for the 8
# m-chunks of a head. 'a' = ACT exact exp, 'd' = DVE integer exp.
# Phase 0 (image-0 heads): ACT also absorbs image-1 qkv/V conversions.
# Phase 1 (image-1 heads): DVE also absorbs proj residual adds.
_PAT_5A3D = ['a', 'd', 'a', 'a', 'd', 'a', 'd', 'a']
_PAT_4A4D = ['a', 'd', 'a', 'd', 'a', 'd', 'a', 'd']
_PAT_6A2D = ['a', 'a', 'd', 'a', 'a', 'd', 'a', 'a']
EXP_PAT = {
    0: [_PAT_4A4D, _PAT_5A3D] * 4,
    1: [_PAT_5A3D, _PAT_6A2D] * 4,
}


def build_nc():
    nc = bass.Bass()
    xm_h = nc.dram_tensor("xm", [BPC, C, N], F8, kind="ExternalInput")
    xr_h = nc.dram_tensor("xr", [BPC, C, N], F32, kind="ExternalInput")
    wqkv_h = nc.dram_tensor("wqkv", [C, 3 * C], F8, kind="ExternalInput")
    pw_h = nc.dram_tensor("pw", [C, C], F8, kind="ExternalInput")
    bq_h = nc.dram_tensor("bq", [128, 4], F32, kind="ExternalInput")
    y_h = nc.dram_tensor("y", [BPC, C, N], F32, kind="ExternalOutput")

    MC = N // 128          # m-chunks (key/value token chunks)
    NH = N // 512          # moving-dim halves
    dma = nc.gpsimd.dma_start

    BUFS = dict(
        xm=4, xr=8, qk=18, vt=9, es=6, on=4, rb1=3, rbc=3, stg=2, y=3,
    )

    with tile.TileContext(nc) as tc:
        with (
            tc.tile_pool(name="w", bufs=1) as wp,
            tc.tile_pool(name="sb", bufs=2) as sb,
            tc.tile_pool(name="ps", bufs=2, space=bass.MemorySpace.PSUM) as ps,
            tc.tile_pool(name="pso", bufs=2, space=bass.MemorySpace.PSUM) as pso,
            tc.tile_pool(name="dr", bufs=4, space=bass.MemorySpace.DRAM) as dr,
        ):
            wqkv_dr = []   # u -> [128, 2*3C] fp8 (k-tile pair packed)
            pw_dr = []     # u -> [128, 2*C] fp8
            xm_dr = {}     # (img, u) -> [128, 2*N] fp8
            xr_sb = {}     # (img, oc) -> [128, N] f32
            qk_sb = {}     # (img, oc) -> [128, N] fp8 (oc 0-3 Q, 4-7 K)
            vt_sb = {}     # (img, j) -> [128, 2*8*65] fp8 (V^T pair + ones col)
            on_dr = {}     # (img, u) -> [128, 2*N] fp8 (normalized attn out)

            def wq_r(u):
                return wqkv_dr[u][:].rearrange("p (i o) -> p i o", i=2)

            def pw_r(u):
                return pw_dr[u][:].rearrange("p (i o) -> p i o", i=2)

            def xm_r(img, u):
                return xm_dr[(img, u)][:].rearrange("p (i n) -> p i n", i=2)

            def on_r(img, u):
                return on_dr[(img, u)][:].rearrange("p (i n) -> p i n", i=2)

            def load_weights():
                for u in range(2):
                    w = wp.tile([128, 2 * 3 * C], F8, tag=f"wqkv{u}",
                                name=f"wqkv{u}")
                    wqkv_dr.append(w)
                    for i in range(2):
                        dma(out=w[:, i * 3 * C:(i + 1) * 3 * C],
                            in_=wqkv_h[(2 * u + i) * 128:(2 * u + i + 1) * 128, :])
                bq = wp.tile([128, 4], F32, tag="bq", name="bq")
                dma(out=bq[:], in_=bq_h[:])
                for u in range(2):
                    w = wp.tile([128, 2 * C], F8, tag=f"pw{u}", name=f"pw{u}")
                    pw_dr.append(w)
                    for i in range(2):
                        dma(out=w[:, i * C:(i + 1) * C],
                            in_=pw_h[(2 * u + i) * 128:(2 * u + i + 1) * 128, :])
                return bq

            def load_xm(img):
                for u in range(2):
                    t = sb.tile([128, 2 * N], F8, tag="xm", bufs=BUFS["xm"],
                                name=f"xm{img}_{u}")
                    for i in range(2):
                        for nh in range(NH):
                            dma(out=t[:, i * N + nh * 512:i * N + (nh + 1) * 512],
                                in_=xm_h[img, (2 * u + i) * 128:(2 * u + i + 1) * 128,
                                         nh * 512:(nh + 1) * 512])
                    xm_dr[(img, u)] = t

            def load_xr(img):
                for oc in range(4):
                    t = sb.tile([128, N], F32, tag="xr", bufs=BUFS["xr"],
                                name=f"xr{img}_{oc}")
                    nc.sync.dma_start(out=t[:], in_=xr_h[img, oc * 128:(oc + 1) * 128, :])
                    xr_sb[(img, oc)] = t

            def emit_qkv(img, ocs):
                # oc 0-3: Q chunks (bias added); oc 4-7: K chunks (no bias).
                for oc in ocs:
                    q_ps = ps.tile([128, N], F32, tag="s")
                    for nh in range(NH):
                        for u in range(2):
                            nc.tensor.matmul(
                                q_ps[:, nh * 512:(nh + 1) * 512],
                                wq_r(u)[:, :, oc * 128:(oc + 1) * 128],
                                xm_r(img, u)[:, :, nh * 512:(nh + 1) * 512],
                                start=(u == 0), stop=(u == 1), perf_mode=DR,
                            )
                    t = sb.tile([128, N], F8, tag="qk", bufs=BUFS["qk"],
                                name=f"qk{img}_{oc}")
                    if oc < 4:
                        nc.scalar.activation(
                            t[:], q_ps[:], mybir.ActivationFunctionType.Identity,
                            bias=bq_sb[:, oc:oc + 1])
                    else:
                        nc.scalar.activation(
                            t[:], q_ps[:], mybir.ActivationFunctionType.Copy)
                    qk_sb[(img, oc)] = t

            def emit_v(img, mcs):
                # V^T [m, c] per m-chunk; pairs packed for the DR attn@V.
                for mc in mcs:
                    v_ps = ps.tile([128, 512], F32, tag="s")
                    for u in range(2):
                        nc.tensor.matmul(
                            v_ps[:],
                            xm_r(img, u)[:, :, mc * 128:(mc + 1) * 128],
                            wq_r(u)[:, :, 2 * C:3 * C],
                            start=(u == 0), stop=(u == 1), perf_mode=DR,
                        )
                    j, slot = mc // 2, mc % 2
                    if slot == 0:
                        t = sb.tile([128, 2 * HEADS * 80], F8, tag="vt",
                                    bufs=BUFS["vt"], name=f"vt{img}_{j}")
                        tv = t[:].rearrange("p (i h u) -> p i h u", i=2, u=80)
                        nc.gpsimd.memset(tv[:, :, :, 64:65], 1.0)
                        vt_sb[(img, j)] = t
                    else:
                        t = vt_sb[(img, j)]
                        tv = t[:].rearrange("p (i h u) -> p i h u", i=2, u=80)
                    nc.scalar.activation(
                        tv[:, slot, :, 0:64],
                        v_ps[:].rearrange("p (h u) -> p h u", u=64),
                        mybir.ActivationFunctionType.Copy)

            def vt_head(img, j, h):
                t = vt_sb[(img, j)][:].rearrange("p (i hu) -> p i hu", i=2)
                return t[:, :, h * 80:h * 80 + 65]

            def emit_head(img, h, phase, pos, filler=None):
                pair, half = h // 2, h % 2
                base = 64 * half
                qt, kt = qk_sb[(img, pair)], qk_sb[(img, 4 + pair)]
                pat = EXP_PAT[phase][pos]
                o_ps = pso.tile([65, N], F32, tag="o")
                es = {}

                def s_step(mc):
                    s_ps = ps.tile([128, N], F32, tag="s")
                    for nh in range(NH):
                        nc.tensor.matmul(
                            s_ps[:, nh * 512:(nh + 1) * 512],
                            kt[base:base + 64, mc * 128:(mc + 1) * 128],
                            qt[base:base + 64, nh * 512:(nh + 1) * 512],
                            start=True, stop=True,
                        )
                    j, slot = mc // 2, mc % 2
                    if slot == 0:
                        es[j] = sb.tile([128, 2 * N], F8, tag="es",
                                        bufs=BUFS["es"], name=f"es{j}")
                    ev = es[j][:, slot * N:(slot + 1) * N]
                    if pat[mc] == 'a':
                        nc.scalar.activation(
                            ev, s_ps[:], mybir.ActivationFunctionType.Exp,
                            scale=SCALE)
                    else:
                        nc.vector.tensor_scalar(
                            ev.bitcast(I8), s_ps[:], EXP_C1, EXP_C2,
                            mybir.AluOpType.mult, mybir.AluOpType.add)

                def o_step(j):
                    er = es[j][:].rearrange("p (i n) -> p i n", i=2)
                    for nh in range(NH):
                        nc.tensor.matmul(
                            o_ps[:, nh * 512:(nh + 1) * 512],
                            vt_head(img, j, h),
                            er[:, :, nh * 512:(nh + 1) * 512],
                            start=(j == 0), stop=(j == 3), perf_mode=DR,
                            skip_group_check=True,
                        )
                    if j > 0:
                        es.pop(j - 1, None)

                s_step(0)
                s_step(1)
                if filler is not None:
                    filler()
                s_step(2)
                s_step(3)
                o_step(0)
                s_step(4)
                s_step(5)
                o_step(1)
                s_step(6)
                s_step(7)
                o_step(2)
                o_step(3)

                # normalize: psum row 64 holds the softmax denominator
                rb1 = sb.tile([1, N], F32, tag="rb1", bufs=BUFS["rb1"])
                nc.vector.reciprocal(rb1[:], o_ps[64:65, :])
                rbc = sb.tile([64, N], F32, tag="rbc", bufs=BUFS["rbc"])
                rd = dr.tile([1, N], F32, tag="rd")
                dma(out=rd[:], in_=rb1[:])
                dma(out=rbc[:], in_=rd[:].partition_broadcast(64))
                u, i = h // 4, (h // 2) % 2
                if half == 0:
                    nc.vector.tensor_mul(
                        on_dr[(img, u)][0:64, i * N:(i + 1) * N],
                        o_ps[0:64, :], rbc[:])
                else:
                    stg = sb.tile([64, N], F8, tag="stg", bufs=BUFS["stg"])
                    nc.vector.tensor_mul(stg[:], o_ps[0:64, :], rbc[:])
                    dma(out=on_dr[(img, u)][64:128, i * N:(i + 1) * N],
                        in_=stg[:])

            def alloc_on(img):
                for u in range(2):
                    on_dr[(img, u)] = sb.tile(
                        [128, 2 * N], F8, tag="on", bufs=BUFS["on"],
                        name=f"on{img}_{u}")

            def emit_proj(img, ocs):
                for oc in ocs:
                    p_ps = ps.tile([128, N], F32, tag="s")
                    for nh in range(NH):
                        for u in range(2):
                            nc.tensor.matmul(
                                p_ps[:, nh * 512:(nh + 1) * 512],
                                pw_r(u)[:, :, oc * 128:(oc + 1) * 128],
                                on_r(img, u)[:, :, nh * 512:(nh + 1) * 512],
                                start=(u == 0), stop=(u == 1), perf_mode=DR,
                            )
                    yt = sb.tile([128, N], F32, tag="y", bufs=BUFS["y"])
                    nc.vector.tensor_add(yt[:], p_ps[:], xr_sb[(img, oc)][:])
                    nc.sync.dma_start(out=y_h[img, oc * 128:(oc + 1) * 128, :],
                                      in_=yt[:])

            # ---------- emission schedule (2 images, pipelined) ----------
            # warm the ACT exp table during the input DMAs
            warm = wp.tile([1, 1], F32, tag="warm", name="warm")
            nc.vector.memset(warm[:], 0.0)
            nc.scalar.activation(
                warm[:], warm[:], mybir.ActivationFunctionType.Exp)
            bq_sb = load_weights()
            load_xm(0)
            alloc_on(0)

            # minimal preamble: first head (h=1) needs Q/K chunk 0 and the
            # first V pair; the rest drains as fillers inside the head loop
            emit_qkv(0, [0, 4])
            emit_v(0, [0, 1, 2, 3])

            head_order = [1, 0, 3, 2, 5, 4, 7, 6]  # odd first: the last
            # normalize of each pair is the direct DVE write, keeping the
            # slow stg-DMA path off the critical edge into proj

            fillers0 = [
                lambda: (emit_qkv(0, [1, 5]), emit_v(0, [4, 5, 6, 7])),
                lambda: emit_qkv(0, [2, 6]),
                lambda: (emit_qkv(0, [3, 7]), load_xm(1)),
                lambda: emit_qkv(1, [0, 4]),
                lambda: (emit_qkv(1, [1, 5]), emit_v(1, [0, 1])),
                lambda: (emit_qkv(1, [2, 6]), emit_v(1, [2, 3])),
                lambda: (emit_qkv(1, [3, 7]), emit_v(1, [4, 5])),
                lambda: (emit_v(1, [6, 7]), load_xr(0), alloc_on(1)),
            ]
            for pos, h in enumerate(head_order):
                emit_head(0, h, 0, pos, filler=fillers0[pos])

            fillers1 = [
                None,
                lambda: emit_proj(0, [0]),
                lambda: emit_proj(0, [1]),
                lambda: emit_proj(0, [2]),
                lambda: emit_proj(0, [3]),
                lambda: load_xr(1),
                None,
                None,
            ]
            for pos, h in enumerate(head_order):
                emit_head(1, h, 1, pos, filler=fillers1[pos])
            emit_proj(1, range(4))

    _split_multi_waits(nc)
    return nc


_CACHE = {}


def _get_nc(mode=None):
    if "nc" not in _CACHE:
        _CACHE["nc"] = build_nc()
    return _CACHE["nc"]


def prepare_inputs(x, qkv_w, qkv_b, proj_w, proj_b):
    f8 = ml_dtypes.float8_e4m3
    x = np.asarray(x, np.float32).reshape(B, C, N)
    qkv_w = np.asarray(qkv_w, np.float32)
    qkv_b = np.asarray(qkv_b, np.float32)
    proj_w = np.asarray(proj_w, np.float32)
    proj_b = np.asarray(proj_b, np.float32)

    xm = np.ascontiguousarray(x.astype(f8))
    # residual with proj bias and the folded V-bias term (sum_m attn == 1)
    rbias = proj_b + proj_w.astype(f8).astype(np.float32) @ qkv_b[2 * C:]
    xr = np.ascontiguousarray(x + rbias[None, :, None])
    wqkv = np.ascontiguousarray(qkv_w.T.astype(f8))
    pw = np.ascontiguousarray(proj_w.T.astype(f8))
    bq = np.ascontiguousarray(qkv_b[:C].reshape(4, 128).T)

    in_maps = []
    for c in range(NCORES):
        sl = slice(c * BPC, (c + 1) * BPC)
        in_maps.append({
            "xm": xm[sl], "xr": xr[sl], "wqkv": wqkv, "pw": pw, "bq": bq,
        })
    return in_maps


def run(x, qkv_w, qkv_b, proj_w, proj_b, mode=None, **spmd_kwargs):
    nc = _get_nc()
    in_maps = prepare_inputs(x, qkv_w, qkv_b, proj_w, proj_b)
    res = run_bass_kernel_spmd(nc, in_maps, list(range(NCORES)), **spmd_kwargs)
    y = np.concatenate([np.asarray(res.results[c]["y"]) for c in range(NCORES)], axis=0)
    return res, y.reshape(B, C, 32, 32).astype(np.float32)


MM_MODE = "fp8dr"


def kernel(x, qkv_w, qkv_b, proj_w, proj_b):
    _, y = run(x, qkv_w, qkv_b, proj_w, proj_b)
    return y


# revision 11
# speedup vs baseline: 1.1853x; 1.0197x over previous
"""Attention2D Trainium2 Bass kernel — fp8 DoubleRow edition.

Reference computation (per batch image, C=512 channels, N=1024 tokens):
    qkv = qkv_w @ x + qkv_b            # (1536, N)
    q,k,v per head (8 heads, head_dim 64)
    attn = softmax(scale * q.T k)      # (N, N) per head, scale = C**-0.5
    out  = v @ attn.T                  # (64, N) per head
    y    = x + proj_w @ out + proj_b

Sharding: data-parallel over batch. 16 images / 8 cores = 2 images per core.
Weights replicated; no collectives.

Numerics/performance strategy (validated offline: rel err ~2.6e-3 vs the
2e-2 gate):
  - Everything quantized to fp8 e4m3. All the big channel-contraction
    matmuls (qkv, V, attn@V, proj) run in MatmulPerfMode.DoubleRow: one
    instruction contracts 2 k-tiles of 128 at 0.5 cycles/col (4x f32r).
    Layout for DR: lhsT [K,2,M], rhs [K,2,N] with the k-tile pair packed
    in the free dim.
  - S = q.T k has contraction 64 only, stays fp8 non-DR (1 cyc/col) with
    the head pair packed on partition halves (base partition 0/64).
  - K bias is dropped entirely: softmax(q.(k~+bk)) == softmax(q.k~ + f(n));
    the per-n term cancels. Q keeps its bias (supplies the bq.k~ term).
  - V bias folded into the residual on the host: sum_m attn = 1, so
    out_att = Vnorm + bv and y gains the constant proj_w@bv, merged into
    xr = x + proj_b + proj_w@bv.
  - exp is the elementwise bottleneck (2*8*1024^2 elems/core); only ACT
    and DVE can read PSUM. Split: ACT runs exact Exp (also absorbs the
    Q/K/V psum->sbuf fp8 conversions; 'exp_and_others' table covers Exp+
    Copy+Identity so no table reloads); DVE runs a Schraudolph-style
    integer exp: i8 = (S*scale*8*log2e) + (56.5-0.35), bitcast int8 ->
    fp8e4m3 (bias 7, 3 mantissa bits). Rounding-mode miscalibration is
    common-mode and cancels through the softmax denominator.
  - Softmax denominator comes free from a ones-column in the V^T tiles
    (row 64 of the attn@V psum). reciprocal on DVE, partition-broadcast
    on GPSIMD (sbuf->sbuf, no DRAM round trip), normalize-mul on DVE.
"""

import math

import numpy as np
import ml_dtypes

import concourse.bass as bass
import concourse.tile as tile
from concourse import mybir
from concourse.bass_utils import run_bass_kernel_spmd

B, C, N = 16, 512, 1024
HEADS, HD = 8, 64
SCALE = float(C) ** -0.5
NCORES = 8
BPC = B // NCORES  # images per core

F32 = mybir.dt.float32
F8 = mybir.dt.float8e4
I8 = mybir.dt.int8

# Schraudolph exp -> fp8e4m3 constants: byte = round(x*SCALE*8*log2(e) + 56.5
# - 0.35). +56 = bias 7 << 3; +0.5 turns trunc into round; -0.35 recenters the
# piecewise-linear 2^frac error.
EXP_C1 = SCALE * 8.0 / math.log(2.0)
EXP_C2 = 56.5 - 0.35

DR = mybir.MatmulPerfMode.DoubleRow


def _split_multi_waits(nc):
    """Walrus codegen rejects instructions carrying more than one semaphore
    wait. Hoist all but the last wait into standalone InstEventSemaphore ops
    just before them (same engine, so per-engine order is preserved)."""
    n_split = 0
    for f in nc.m.functions:
        for b in f.blocks:
            out = []
            changed = False
            for inst in b.instructions:
                si = inst.sync_info
                waits = list(si.on_wait) if si is not None else []
                if len(waits) > 1:
                    for k, w in enumerate(waits[:-1]):
                        wi = mybir.InstEventSemaphore(
                            name=f"{inst.name}-presync{k}", ins=[], outs=[],
                            sync_info=mybir.SyncInfo(on_wait=[w], on_update=[]),
                        )
                        wi.engine = inst.engine
                        out.append(wi)
                        n_split += 1
                    inst.sync_info = mybir.SyncInfo(
                        on_wait=[waits[-1]], on_update=list(si.on_update)
                    )
                    changed = True
                out.append(inst)
            if changed:
                b.instructions = out
    return n_split


# Per-(phase, head-position) exp engine split: list of 8 engines for the 8
# m-chunks of a head. 'a' = ACT exact exp, 'd' = DVE integer exp.
# Phase 0 (image-0 heads): ACT also absorbs image-1 qkv/V conversions.
# Phase 1 (image-1 heads): DVE also absorbs proj residual adds.
_PAT_5A3D = ['a', 'd', 'a', 'a', 'd', 'a', 'd', 'a']
_PAT_4A4D = ['a', 'd', 'a', 'd', 'a', 'd', 'a', 'd']
_PAT_6A2D = ['a', 'a', 'd', 'a', 'a', 'd', 'a', 'a']
_PAT_7A1D = ['a', 'a', 'a', 'd', 'a', 'a', 'a', 'a']
_PAT_8A0D = ['a'] * 8
EXP_PAT = {
    0: [_PAT_4A4D, _PAT_5A3D] * 4,
    # tail is DVE-bound (proj adds + last norms): hand the late exps to ACT
    1: [_PAT_4A4D, _PAT_4A4D, _PAT_4A4D, _PAT_5A3D,
        _PAT_5A3D, _PAT_6A2D, _PAT_8A0D, _PAT_8A0D],
}


def build_nc():
    nc = bass.Bass()
    xm_h = nc.dram_tensor("xm", [BPC, C, N], F8, kind="ExternalInput")
    xr_h = nc.dram_tensor("xr", [BPC, C, N], F32, kind="ExternalInput")
    wqkv_h = nc.dram_tensor("wqkv", [C, 3 * C], F8, kind="ExternalInput")
    pw_h = nc.dram_tensor("pw", [C, C], F8, kind="ExternalInput")
    bq_h = nc.dram_tensor("bq", [128, 4], F32, kind="ExternalInput")
    y_h = nc.dram_tensor("y", [BPC, C, N], F32, kind="ExternalOutput")

    MC = N // 128          # m-chunks (key/value token chunks)
    NH = N // 512          # moving-dim halves
    dma = nc.gpsimd.dma_start

    BUFS = dict(
        xm=4, xr=8, qk=18, vt=9, es=8, on=4, rb1=3, rbc=3, stg=2, y=3,
    )

    with tile.TileContext(nc) as tc:
        with (
            tc.tile_pool(name="w", bufs=1) as wp,
            tc.tile_pool(name="sb", bufs=2) as sb,
            tc.tile_pool(name="ps", bufs=2, space=bass.MemorySpace.PSUM) as ps,
            tc.tile_pool(name="pso", bufs=2, space=bass.MemorySpace.PSUM) as pso,
            tc.tile_pool(name="dr", bufs=4, space=bass.MemorySpace.DRAM) as dr,
        ):
            wqkv_dr = []   # u -> [128, 2*3C] fp8 (k-tile pair packed)
            pw_dr = []     # u -> [128, 2*C] fp8
            xm_dr = {}     # (img, u) -> [128, 2*N] fp8
            xr_sb = {}     # (img, oc) -> [128, N] f32
            qk_sb = {}     # (img, oc) -> [128, N] fp8 (oc 0-3 Q, 4-7 K)
            vt_sb = {}     # (img, j) -> [128, 2*8*65] fp8 (V^T pair + ones col)
            on_dr = {}     # (img, u) -> [128, 2*N] fp8 (normalized attn out)

            def wq_r(u):
                return wqkv_dr[u][:].rearrange("p (i o) -> p i o", i=2)

            def pw_r(u):
                return pw_dr[u][:].rearrange("p (i o) -> p i o", i=2)

            def xm_r(img, u):
                return xm_dr[(img, u)][:].rearrange("p (i n) -> p i n", i=2)

            def on_r(img, u):
                return on_dr[(img, u)][:].rearrange("p (i n) -> p i n", i=2)

            def load_weights():
                # weights on the gpsimd queue, image-0 x on the sync queue,
                # interleaved in first-use order so the first qkv matmul
                # unblocks after a couple of transfers per queue
                for u in range(2):
                    w = wp.tile([128, 2 * 3 * C], F8, tag=f"wqkv{u}",
                                name=f"wqkv{u}")
                    wqkv_dr.append(w)
                    t = sb.tile([128, 2 * N], F8, tag="xm", bufs=BUFS["xm"],
                                name=f"xm0_{u}")
                    xm_dr[(0, u)] = t
                    for i in range(2):
                        dma(out=w[:, i * 3 * C:(i + 1) * 3 * C],
                            in_=wqkv_h[(2 * u + i) * 128:(2 * u + i + 1) * 128, :])
                    if u == 0:
                        bq = wp.tile([128, 4], F32, tag="bq", name="bq")
                        dma(out=bq[:], in_=bq_h[:])
                    for nh in range(NH):
                        for i in range(2):
                            nc.sync.dma_start(
                                out=t[:, i * N + nh * 512:i * N + (nh + 1) * 512],
                                in_=xm_h[0, (2 * u + i) * 128:(2 * u + i + 1) * 128,
                                         nh * 512:(nh + 1) * 512])
                for u in range(2):
                    w = wp.tile([128, 2 * C], F8, tag=f"pw{u}", name=f"pw{u}")
                    pw_dr.append(w)
                    for i in range(2):
                        dma(out=w[:, i * C:(i + 1) * C],
                            in_=pw_h[(2 * u + i) * 128:(2 * u + i + 1) * 128, :])
                return bq

            def load_xm(img):
                for u in range(2):
                    t = sb.tile([128, 2 * N], F8, tag="xm", bufs=BUFS["xm"],
                                name=f"xm{img}_{u}")
                    for i in range(2):
                        for nh in range(NH):
                            dma(out=t[:, i * N + nh * 512:i * N + (nh + 1) * 512],
                                in_=xm_h[img, (2 * u + i) * 128:(2 * u + i + 1) * 128,
                                         nh * 512:(nh + 1) * 512])
                    xm_dr[(img, u)] = t

            def load_xr(img):
                for oc in range(4):
                    t = sb.tile([128, N], F32, tag="xr", bufs=BUFS["xr"],
                                name=f"xr{img}_{oc}")
                    nc.sync.dma_start(out=t[:], in_=xr_h[img, oc * 128:(oc + 1) * 128, :])
                    xr_sb[(img, oc)] = t

            def emit_qkv(img, ocs):
                # oc 0-3: Q chunks (bias added); oc 4-7: K chunks (no bias).
                for oc in ocs:
                    q_ps = ps.tile([128, N], F32, tag="s")
                    for nh in range(NH):
                        for u in range(2):
                            nc.tensor.matmul(
                                q_ps[:, nh * 512:(nh + 1) * 512],
                                wq_r(u)[:, :, oc * 128:(oc + 1) * 128],
                                xm_r(img, u)[:, :, nh * 512:(nh + 1) * 512],
                                start=(u == 0), stop=(u == 1), perf_mode=DR,
                            )
                    t = sb.tile([128, N], F8, tag="qk", bufs=BUFS["qk"],
                                name=f"qk{img}_{oc}")
                    if oc < 4:
                        nc.scalar.activation(
                            t[:], q_ps[:], mybir.ActivationFunctionType.Identity,
                            bias=bq_sb[:, oc:oc + 1])
                    else:
                        nc.scalar.activation(
                            t[:], q_ps[:], mybir.ActivationFunctionType.Copy)
                    qk_sb[(img, oc)] = t

            def emit_v(img, mcs):
                # V^T [m, c] per m-chunk; pairs packed for the DR attn@V.
                for mc in mcs:
                    v_ps = ps.tile([128, 512], F32, tag="s")
                    for u in range(2):
                        nc.tensor.matmul(
                            v_ps[:],
                            xm_r(img, u)[:, :, mc * 128:(mc + 1) * 128],
                            wq_r(u)[:, :, 2 * C:3 * C],
                            start=(u == 0), stop=(u == 1), perf_mode=DR,
                        )
                    j, slot = mc // 2, mc % 2
                    if slot == 0:
                        t = sb.tile([128, 2 * HEADS * 80], F8, tag="vt",
                                    bufs=BUFS["vt"], name=f"vt{img}_{j}")
                        tv = t[:].rearrange("p (i h u) -> p i h u", i=2, u=80)
                        nc.gpsimd.memset(tv[:, :, :, 64:65], 1.0)
                        vt_sb[(img, j)] = t
                    else:
                        t = vt_sb[(img, j)]
                        tv = t[:].rearrange("p (i h u) -> p i h u", i=2, u=80)
                    nc.scalar.activation(
                        tv[:, slot, :, 0:64],
                        v_ps[:].rearrange("p (h u) -> p h u", u=64),
                        mybir.ActivationFunctionType.Copy)

            def vt_head(img, j, h):
                t = vt_sb[(img, j)][:].rearrange("p (i hu) -> p i hu", i=2)
                return t[:, :, h * 80:h * 80 + 65]

            def emit_head(img, h, phase, pos, filler=None):
                pair, half = h // 2, h % 2
                base = 64 * half
                qt, kt = qk_sb[(img, pair)], qk_sb[(img, 4 + pair)]
                pat = EXP_PAT[phase][pos]
                o_ps = pso.tile([65, N], F32, tag="o")
                es = {}

                def s_step(mc):
                    s_ps = ps.tile([128, N], F32, tag="s")
                    for nh in range(NH):
                        nc.tensor.matmul(
                            s_ps[:, nh * 512:(nh + 1) * 512],
                            kt[base:base + 64, mc * 128:(mc + 1) * 128],
                            qt[base:base + 64, nh * 512:(nh + 1) * 512],
                            start=True, stop=True,
                        )
                    j, slot = mc // 2, mc % 2
                    if slot == 0:
                        es[j] = sb.tile([128, 2 * N], F8, tag="es",
                                        bufs=BUFS["es"], name=f"es{j}")
                    ev = es[j][:, slot * N:(slot + 1) * N]
                    if pat[mc] == 'a':
                        nc.scalar.activation(
                            ev, s_ps[:], mybir.ActivationFunctionType.Exp,
                            scale=SCALE)
                    else:
                        nc.vector.tensor_scalar(
                            ev.bitcast(I8), s_ps[:], EXP_C1, EXP_C2,
                            mybir.AluOpType.mult, mybir.AluOpType.add)

                def o_step(j):
                    er = es[j][:].rearrange("p (i n) -> p i n", i=2)
                    for nh in range(NH):
                        nc.tensor.matmul(
                            o_ps[:, nh * 512:(nh + 1) * 512],
                            vt_head(img, j, h),
                            er[:, :, nh * 512:(nh + 1) * 512],
                            start=(j == 0), stop=(j == 3), perf_mode=DR,
                            skip_group_check=True,
                        )
                    if j > 0:
                        es.pop(j - 1, None)

                fl = list(filler) if filler else []
                fl += [None] * (3 - len(fl))
                s_step(0)
                s_step(1)
                if fl[0]:
                    fl[0]()
                s_step(2)
                s_step(3)
                if fl[1]:
                    fl[1]()
                o_step(0)
                s_step(4)
                s_step(5)
                if fl[2]:
                    fl[2]()
                o_step(1)
                s_step(6)
                s_step(7)
                o_step(2)
                o_step(3)

                # normalize: psum row 64 holds the softmax denominator
                rb1 = sb.tile([1, N], F32, tag="rb1", bufs=BUFS["rb1"])
                nc.vector.reciprocal(rb1[:], o_ps[64:65, :])
                rbc = sb.tile([64, N], F32, tag="rbc", bufs=BUFS["rbc"])
                rd = dr.tile([1, N], F32, tag="rd")
                dma(out=rd[:], in_=rb1[:])
                dma(out=rbc[:], in_=rd[:].partition_broadcast(64))
                u, i = h // 4, (h // 2) % 2
                if half == 0:
                    nc.vector.tensor_mul(
                        on_dr[(img, u)][0:64, i * N:(i + 1) * N],
                        o_ps[0:64, :], rbc[:])
                else:
                    stg = sb.tile([64, N], F8, tag="stg", bufs=BUFS["stg"])
                    nc.vector.tensor_mul(stg[:], o_ps[0:64, :], rbc[:])
                    dma(out=on_dr[(img, u)][64:128, i * N:(i + 1) * N],
                        in_=stg[:])

            def alloc_on(img):
                for u in range(2):
                    on_dr[(img, u)] = sb.tile(
                        [128, 2 * N], F8, tag="on", bufs=BUFS["on"],
                        name=f"on{img}_{u}")

            def emit_proj(img, ocs):
                for oc in ocs:
                    p_ps = ps.tile([128, N], F32, tag="s")
                    for nh in range(NH):
                        for u in range(2):
                            nc.tensor.matmul(
                                p_ps[:, nh * 512:(nh + 1) * 512],
                                pw_r(u)[:, :, oc * 128:(oc + 1) * 128],
                                on_r(img, u)[:, :, nh * 512:(nh + 1) * 512],
                                start=(u == 0), stop=(u == 1), perf_mode=DR,
                            )
                    yt = sb.tile([128, N], F32, tag="y", bufs=BUFS["y"])
                    nc.vector.tensor_add(yt[:], p_ps[:], xr_sb[(img, oc)][:])
                    nc.sync.dma_start(out=y_h[img, oc * 128:(oc + 1) * 128, :],
                                      in_=yt[:])

            # ---------- emission schedule (2 images, pipelined) ----------
            # warm the ACT exp table during the input DMAs
            warm = wp.tile([1, 1], F32, tag="warm", name="warm")
            nc.vector.memset(warm[:], 0.0)
            nc.scalar.activation(
                warm[:], warm[:], mybir.ActivationFunctionType.Exp)
            bq_sb = load_weights()
            alloc_on(0)

            # minimal preamble: first head (h=1) needs Q/K chunk 0 and the
            # first V pair; the rest drains as fillers inside the head loop
            emit_qkv(0, [0, 4])
            emit_v(0, [0, 1, 2, 3])

            head_order = [1, 0, 3, 2, 5, 4, 7, 6]  # odd first: the last
            # normalize of each pair is the direct DVE write, keeping the
            # slow stg-DMA path off the critical edge into proj

            fillers0 = [
                [lambda: emit_qkv(0, [1]),
                 lambda: (emit_qkv(0, [5]), emit_v(0, [4, 5])),
                 lambda: emit_v(0, [6, 7])],
                [lambda: emit_qkv(0, [2]), lambda: emit_qkv(0, [6])],
                [lambda: emit_qkv(0, [3]),
                 lambda: emit_qkv(0, [7]),
                 lambda: load_xm(1)],
                [lambda: emit_qkv(1, [0]), lambda: emit_qkv(1, [4])],
                [lambda: emit_qkv(1, [1]),
                 lambda: emit_qkv(1, [5]),
                 lambda: emit_v(1, [0, 1])],
                [lambda: emit_qkv(1, [2]),
                 lambda: emit_qkv(1, [6]),
                 lambda: emit_v(1, [2, 3])],
                [lambda: emit_qkv(1, [3]),
                 lambda: emit_qkv(1, [7]),
                 lambda: emit_v(1, [4, 5])],
                [lambda: emit_v(1, [6, 7]),
                 lambda: (load_xr(0), alloc_on(1))],
            ]
            for pos, h in enumerate(head_order):
                emit_head(0, h, 0, pos, filler=fillers0[pos])

            fillers1 = [
                None,
                [None, lambda: emit_proj(0, [0])],
                [None, lambda: emit_proj(0, [1])],
                [None, lambda: emit_proj(0, [2])],
                [None, lambda: emit_proj(0, [3])],
                [lambda: load_xr(1)],
                None,
                None,
            ]
            for pos, h in enumerate(head_order):
                emit_head(1, h, 1, pos, filler=fillers1[pos])
            emit_proj(1, range(4))

    _split_multi_waits(nc)
    return nc


_CACHE = {}


def _get_nc(mode=None):
    if "nc" not in _CACHE:
        _CACHE["nc"] = build_nc()
    return _CACHE["nc"]


def prepare_inputs(x, qkv_w, qkv_b, proj_w, proj_b):
    f8 = ml_dtypes.float8_e4m3
    x = np.asarray(x, np.float32).reshape(B, C, N)
    qkv_w = np.asarray(qkv_w, np.float32)
    qkv_b = np.asarray(qkv_b, np.float32)
    proj_w = np.asarray(proj_w, np.float32)
    proj_b = np.asarray(proj_b, np.float32)

    xm = np.ascontiguousarray(x.astype(f8))
    # residual with proj bias and the folded V-bias term (sum_m attn == 1)
    rbias = proj_b + proj_w.astype(f8).astype(np.float32) @ qkv_b[2 * C:]
    xr = np.ascontiguousarray(x + rbias[None, :, None])
    wqkv = np.ascontiguousarray(qkv_w.T.astype(f8))
    pw = np.ascontiguousarray(proj_w.T.astype(f8))
    bq = np.ascontiguousarray(qkv_b[:C].reshape(4, 128).T)

    in_maps = []
    for c in range(NCORES):
        sl = slice(c * BPC, (c + 1) * BPC)
        in_maps.append({
            "xm": xm[sl], "xr": xr[sl], "wqkv": wqkv, "pw": pw, "bq": bq,
        })
    return in_maps


def run(x, qkv_w, qkv_b, proj_w, proj_b, mode=None, **spmd_kwargs):
    nc = _get_nc()
    in_maps = prepare_inputs(x, qkv_w, qkv_b, proj_w, proj_b)
    res = run_bass_kernel_spmd(nc, in_maps, list(range(NCORES)), **spmd_kwargs)
    y = np.concatenate([np.asarray(res.results[c]["y"]) for c in range(NCORES)], axis=0)
    return res, y.reshape(B, C, 32, 32).astype(np.float32)


MM_MODE = "fp8dr"


def kernel(x, qkv_w, qkv_b, proj_w, proj_b):
    _, y = run(x, qkv_w, qkv_b, proj_w, proj_b)
    return y


# revision 15
# speedup vs baseline: 1.2501x; 1.0546x over previous
"""Attention2D Trainium2 Bass kernel — fp8 DoubleRow edition.

Reference computation (per batch image, C=512 channels, N=1024 tokens):
    qkv = qkv_w @ x + qkv_b            # (1536, N)
    q,k,v per head (8 heads, head_dim 64)
    attn = softmax(scale * q.T k)      # (N, N) per head, scale = C**-0.5
    out  = v @ attn.T                  # (64, N) per head
    y    = x + proj_w @ out + proj_b

Sharding: data-parallel over batch. 16 images / 8 cores = 2 images per core.
Weights replicated; no collectives.

Numerics/performance strategy (validated offline: rel err ~2.6e-3 vs the
2e-2 gate):
  - Everything quantized to fp8 e4m3. All the big channel-contraction
    matmuls (qkv, V, attn@V, proj) run in MatmulPerfMode.DoubleRow: one
    instruction contracts 2 k-tiles of 128 at 0.5 cycles/col (4x f32r).
    Layout for DR: lhsT [K,2,M], rhs [K,2,N] with the k-tile pair packed
    in the free dim.
  - S = q.T k has contraction 64 only, stays fp8 non-DR (1 cyc/col) with
    the head pair packed on partition halves (base partition 0/64).
  - K bias is dropped entirely: softmax(q.(k~+bk)) == softmax(q.k~ + f(n));
    the per-n term cancels. Q keeps its bias (supplies the bq.k~ term).
  - V bias folded into the residual on the host: sum_m attn = 1, so
    out_att = Vnorm + bv and y gains the constant proj_w@bv, merged into
    xr = x + proj_b + proj_w@bv.
  - exp is the elementwise bottleneck (2*8*1024^2 elems/core); only ACT
    and DVE can read PSUM. Split: ACT runs exact Exp (also absorbs the
    Q/K/V psum->sbuf fp8 conversions; 'exp_and_others' table covers Exp+
    Copy+Identity so no table reloads); DVE runs a Schraudolph-style
    integer exp: i8 = (S*scale*8*log2e) + (56.5-0.35), bitcast int8 ->
    fp8e4m3 (bias 7, 3 mantissa bits). Rounding-mode miscalibration is
    common-mode and cancels through the softmax denominator.
  - Softmax denominator comes free from a ones-column in the V^T tiles
    (row 64 of the attn@V psum). reciprocal on DVE, partition-broadcast
    on GPSIMD (sbuf->sbuf, no DRAM round trip), normalize-mul on DVE.
"""

import math

import numpy as np
import ml_dtypes

import concourse.bass as bass
import concourse.tile as tile
from concourse import mybir
from concourse.bass_utils import run_bass_kernel_spmd

B, C, N = 16, 512, 1024
HEADS, HD = 8, 64
SCALE = float(C) ** -0.5
NCORES = 8
BPC = B // NCORES  # images per core

F32 = mybir.dt.float32
F8 = mybir.dt.float8e4
BF16 = mybir.dt.bfloat16
I8 = mybir.dt.int8

# 'div_psum': tensor_scalar divide with the denominator column read straight
# from PSUM; 'rcp_mult': copy D -> sbuf, reciprocal [128,8], per-chunk mults.
NORM_MODE = "rcp_mult"

# Schraudolph exp -> fp8e4m3 constants: byte = round(x*SCALE*8*log2(e) + 56.5
# - 0.35). +56 = bias 7 << 3; +0.5 turns trunc into round; -0.35 recenters the
# piecewise-linear 2^frac error.
EXP_C1 = SCALE * 8.0 / math.log(2.0)
EXP_C2 = 56.5 - 0.35

DR = mybir.MatmulPerfMode.DoubleRow


def _split_multi_waits(nc):
    """Walrus codegen rejects instructions carrying more than one semaphore
    wait. Hoist all but the last wait into standalone InstEventSemaphore ops
    just before them (same engine, so per-engine order is preserved)."""
    n_split = 0
    for f in nc.m.functions:
        for b in f.blocks:
            out = []
            changed = False
            for inst in b.instructions:
                si = inst.sync_info
                waits = list(si.on_wait) if si is not None else []
                if len(waits) > 1:
                    for k, w in enumerate(waits[:-1]):
                        wi = mybir.InstEventSemaphore(
                            name=f"{inst.name}-presync{k}", ins=[], outs=[],
                            sync_info=mybir.SyncInfo(on_wait=[w], on_update=[]),
                        )
                        wi.engine = inst.engine
                        out.append(wi)
                        n_split += 1
                    inst.sync_info = mybir.SyncInfo(
                        on_wait=[waits[-1]], on_update=list(si.on_update)
                    )
                    changed = True
                out.append(inst)
            if changed:
                b.instructions = out
    return n_split


# Per-(phase, head-position) exp engine split: list of 8 engines for the 8
# m-chunks of a head. 'a' = ACT exact exp, 'd' = DVE integer exp.
# Phase 0 (image-0 heads): ACT also absorbs image-1 qkv/V conversions.
# Phase 1 (image-1 heads): DVE also absorbs proj residual adds.
_PAT_5A3D = ['a', 'd', 'a', 'a', 'd', 'a', 'd', 'a']
_PAT_4A4D = ['a', 'd', 'a', 'd', 'a', 'd', 'a', 'd']
_PAT_6A2D = ['a', 'a', 'd', 'a', 'a', 'd', 'a', 'a']
_PAT_7A1D = ['a', 'a', 'a', 'd', 'a', 'a', 'a', 'a']
_PAT_8A0D = ['a'] * 8
EXP_PAT = {
    0: [_PAT_4A4D, _PAT_5A3D] * 4,
    # tail is DVE-bound (proj adds + last norms): hand the late exps to ACT
    1: [_PAT_4A4D, _PAT_4A4D, _PAT_4A4D, _PAT_5A3D,
        _PAT_5A3D, _PAT_6A2D, _PAT_8A0D, _PAT_8A0D],
}


def build_nc():
    nc = bass.Bass()
    xm_h = nc.dram_tensor("xm", [BPC, C, N], F8, kind="ExternalInput")
    xr_h = nc.dram_tensor("xr", [BPC, C, N], F32, kind="ExternalInput")
    wqkv_h = nc.dram_tensor("wqkv", [C, 3 * C], F8, kind="ExternalInput")
    pw_h = nc.dram_tensor("pw", [C, C], BF16, kind="ExternalInput")
    bq_h = nc.dram_tensor("bq", [128, 4], F32, kind="ExternalInput")
    y_h = nc.dram_tensor("y", [BPC, C, N], F32, kind="ExternalOutput")

    MC = N // 128          # m-chunks (key/value token chunks)
    NH = N // 512          # moving-dim halves
    dma = nc.gpsimd.dma_start

    BUFS = dict(
        xm=4, xr=8, qk=18, vt=9, es=10, on=4, onT=4, dd=3, y=3,
    )

    with tile.TileContext(nc) as tc:
        with (
            tc.tile_pool(name="w", bufs=1) as wp,
            tc.tile_pool(name="sb", bufs=2) as sb,
            tc.tile_pool(name="ps", bufs=2, space=bass.MemorySpace.PSUM) as ps,
            tc.tile_pool(name="pso", bufs=2, space=bass.MemorySpace.PSUM) as pso,
            tc.tile_pool(name="dr", bufs=4, space=bass.MemorySpace.DRAM) as dr,
        ):
            wqkv_dr = []   # u -> [128, 2*3C] fp8 (k-tile pair packed)
            pw_dr = []     # u -> [128, 2*C] fp8
            xm_dr = {}     # (img, u) -> [128, 2*N] fp8
            xr_sb = {}     # (img, oc) -> [128, N] f32
            qk_sb = {}     # (img, oc) -> [128, N] fp8 (oc 0-3 Q, 4-7 K)
            vt_sb = {}     # (img, j) -> [128, 2*8*65] fp8 (V^T pair + ones col)
            on_dr = {}     # (img, u) -> [128, 2*N] bf16 (normalized attn out)
            onT2 = {}      # (img, pair) -> [128, 8*128] bf16 (O^T, pre-transpose)

            def wq_r(u):
                return wqkv_dr[u][:].rearrange("p (i o) -> p i o", i=2)

            def xm_r(img, u):
                return xm_dr[(img, u)][:].rearrange("p (i n) -> p i n", i=2)

            def on_r(img, u):
                return on_dr[(img, u)][:].rearrange("p (i n) -> p i n", i=2)

            def load_weights():
                # weights on the gpsimd queue, image-0 x on the sync queue,
                # interleaved in first-use order so the first qkv matmul
                # unblocks after a couple of transfers per queue
                for u in range(2):
                    w = wp.tile([128, 2 * 3 * C], F8, tag=f"wqkv{u}",
                                name=f"wqkv{u}")
                    wqkv_dr.append(w)
                    t = sb.tile([128, 2 * N], F8, tag="xm", bufs=BUFS["xm"],
                                name=f"xm0_{u}")
                    xm_dr[(0, u)] = t
                    for i in range(2):
                        dma(out=w[:, i * 3 * C:(i + 1) * 3 * C],
                            in_=wqkv_h[(2 * u + i) * 128:(2 * u + i + 1) * 128, :])
                    if u == 0:
                        bq = wp.tile([128, 4], F32, tag="bq", name="bq")
                        dma(out=bq[:], in_=bq_h[:])
                    for nh in range(NH):
                        for i in range(2):
                            nc.sync.dma_start(
                                out=t[:, i * N + nh * 512:i * N + (nh + 1) * 512],
                                in_=xm_h[0, (2 * u + i) * 128:(2 * u + i + 1) * 128,
                                         nh * 512:(nh + 1) * 512])
                for cc in range(4):
                    w = wp.tile([128, C], BF16, tag=f"pw{cc}", name=f"pw{cc}")
                    pw_dr.append(w)
                    dma(out=w[:], in_=pw_h[cc * 128:(cc + 1) * 128, :])
                return bq

            def load_xm(img):
                for u in range(2):
                    t = sb.tile([128, 2 * N], F8, tag="xm", bufs=BUFS["xm"],
                                name=f"xm{img}_{u}")
                    for i in range(2):
                        for nh in range(NH):
                            dma(out=t[:, i * N + nh * 512:i * N + (nh + 1) * 512],
                                in_=xm_h[img, (2 * u + i) * 128:(2 * u + i + 1) * 128,
                                         nh * 512:(nh + 1) * 512])
                    xm_dr[(img, u)] = t

            def load_xr(img):
                for oc in range(4):
                    t = sb.tile([128, N], F32, tag="xr", bufs=BUFS["xr"],
                                name=f"xr{img}_{oc}")
                    nc.sync.dma_start(out=t[:], in_=xr_h[img, oc * 128:(oc + 1) * 128, :])
                    xr_sb[(img, oc)] = t

            def emit_qkv(img, ocs):
                # oc 0-3: Q chunks (bias added); oc 4-7: K chunks (no bias).
                for oc in ocs:
                    q_ps = ps.tile([128, N], F32, tag="s")
                    for nh in range(NH):
                        for u in range(2):
                            nc.tensor.matmul(
                                q_ps[:, nh * 512:(nh + 1) * 512],
                                wq_r(u)[:, :, oc * 128:(oc + 1) * 128],
                                xm_r(img, u)[:, :, nh * 512:(nh + 1) * 512],
                                start=(u == 0), stop=(u == 1), perf_mode=DR,
                            )
                    t = sb.tile([128, N], F8, tag="qk", bufs=BUFS["qk"],
                                name=f"qk{img}_{oc}")
                    if oc < 4:
                        nc.scalar.activation(
                            t[:], q_ps[:], mybir.ActivationFunctionType.Identity,
                            bias=bq_sb[:, oc:oc + 1])
                    else:
                        nc.scalar.activation(
                            t[:], q_ps[:], mybir.ActivationFunctionType.Copy)
                    qk_sb[(img, oc)] = t

            def emit_v(img, mcs):
                # V^T [m, c] per m-chunk; pairs packed for the DR attn@V.
                for mc in mcs:
                    v_ps = ps.tile([128, 512], F32, tag="s")
                    for u in range(2):
                        nc.tensor.matmul(
                            v_ps[:],
                            xm_r(img, u)[:, :, mc * 128:(mc + 1) * 128],
                            wq_r(u)[:, :, 2 * C:3 * C],
                            start=(u == 0), stop=(u == 1), perf_mode=DR,
                        )
                    j, slot = mc // 2, mc % 2
                    if slot == 0:
                        t = sb.tile([128, 2 * HEADS * 80], F8, tag="vt",
                                    bufs=BUFS["vt"], name=f"vt{img}_{j}")
                        tv = t[:].rearrange("p (i h u) -> p i h u", i=2, u=80)
                        nc.gpsimd.memset(tv[:, :, :, 64:65], 1.0)
                        vt_sb[(img, j)] = t
                    else:
                        t = vt_sb[(img, j)]
                        tv = t[:].rearrange("p (i h u) -> p i h u", i=2, u=80)
                    nc.scalar.activation(
                        tv[:, slot, :, 0:64],
                        v_ps[:].rearrange("p (h u) -> p h u", u=64),
                        mybir.ActivationFunctionType.Copy)

            def vt_head(img, h):
                out = []
                for j in range(4):
                    t = vt_sb[(img, j)][:].rearrange("p (i hu) -> p i hu", i=2)
                    out.append(t[:, :, h * 80:h * 80 + 65])
                return out

            def emit_head(img, h, phase, pos, filler=None):
                pair, half = h // 2, h % 2
                base = 64 * half
                qt, kt = qk_sb[(img, pair)], qk_sb[(img, 4 + pair)]
                pat = EXP_PAT[phase][pos]
                # O^T [n, c]: partitions = tokens of each 128-chunk, free =
                # [nc, 65] with the softmax denominator in column 64 (from
                # the ones column of vt). es is the stationary operand.
                o_ps = pso.tile([128, 8 * 65], F32, tag="o")
                opr = o_ps[:].rearrange("p (a u) -> p a u", u=65)
                es = {}

                def s_step(mc):
                    s_ps = ps.tile([128, N], F32, tag="s")
                    for nh in range(NH):
                        nc.tensor.matmul(
                            s_ps[:, nh * 512:(nh + 1) * 512],
                            kt[base:base + 64, mc * 128:(mc + 1) * 128],
                            qt[base:base + 64, nh * 512:(nh + 1) * 512],
                            start=True, stop=True,
                        )
                    j, slot = mc // 2, mc % 2
                    if slot == 0:
                        es[j] = sb.tile([128, 2 * N], F8, tag="es",
                                        bufs=BUFS["es"], name=f"es{j}")
                    ev = es[j][:, slot * N:(slot + 1) * N]
                    if pat[mc] == 'a':
                        nc.scalar.activation(
                            ev, s_ps[:], mybir.ActivationFunctionType.Exp,
                            scale=SCALE)
                    else:
                        nc.vector.tensor_scalar(
                            ev.bitcast(I8), s_ps[:], EXP_C1, EXP_C2,
                            mybir.AluOpType.mult, mybir.AluOpType.add)

                def o_chunks():
                    ers = [es[j][:].rearrange("p (i n) -> p i n", i=2)
                           for j in range(4)]
                    vh = vt_head(img, h)
                    for a in range(8):
                        for j in range(4):
                            nc.tensor.matmul(
                                opr[:, a, :],
                                ers[j][:, :, a * 128:(a + 1) * 128],
                                vh[j],
                                start=(j == 0), stop=(j == 3), perf_mode=DR,
                                skip_group_check=True,
                            )

                for mc in range(MC):
                    s_step(mc)
                    if mc == 1 and filler and len(filler) > 0 and filler[0]:
                        filler[0]()
                    if mc == 3 and filler and len(filler) > 1 and filler[1]:
                        filler[1]()
                    if mc == 5 and filler and len(filler) > 2 and filler[2]:
                        filler[2]()
                o_chunks()

                # normalize O^T by the denominator column, write the bf16
                # head-pair tile consumed by the transpose DMAs
                if half == 0:
                    onT2[(img, pair)] = sb.tile(
                        [128, 8 * 128], BF16, tag="onT", bufs=BUFS["onT"],
                        name=f"onT{img}_{pair}")
                tv = onT2[(img, pair)][:].rearrange("p (a c) -> p a c", c=128)
                if NORM_MODE == "div_psum":
                    for a in range(8):
                        nc.vector.tensor_scalar(
                            tv[:, a, base:base + 64],
                            opr[:, a, 0:64],
                            opr[:, a:a + 1, 64:65], None,
                            mybir.AluOpType.divide)
                else:
                    dd = sb.tile([128, 8], F32, tag="dd", bufs=BUFS["dd"])
                    ddr = dd[:].rearrange("p (a u) -> p a u", u=1)
                    nc.vector.tensor_copy(ddr[:], opr[:, :, 64:65])
                    nc.vector.reciprocal(dd[:], dd[:])
                    for a in range(8):
                        nc.vector.tensor_scalar(
                            tv[:, a, base:base + 64],
                            opr[:, a, 0:64],
                            dd[:, a:a + 1], None,
                            mybir.AluOpType.mult)

                if half == 1:
                    # both halves of the channel pair are in: transpose
                    # [token, channel] -> [channel, token] into the proj rhs
                    u, i = pair // 2, pair % 2
                    for a in range(8):
                        nc.sync.dma_start_transpose(
                            out=on_dr[(img, u)][:, i * N + a * 128:
                                                i * N + (a + 1) * 128],
                            in_=tv[:, a, :])

            def alloc_on(img):
                for u in range(2):
                    on_dr[(img, u)] = sb.tile(
                        [128, 2 * N], BF16, tag="on", bufs=BUFS["on"],
                        name=f"on{img}_{u}")

            def emit_proj(img, ocs):
                for oc in ocs:
                    p_ps = ps.tile([128, N], F32, tag="s")
                    for nh in range(NH):
                        for cc in range(4):
                            u, i = cc // 2, cc % 2
                            nc.tensor.matmul(
                                p_ps[:, nh * 512:(nh + 1) * 512],
                                pw_dr[cc][:, oc * 128:(oc + 1) * 128],
                                on_dr[(img, u)][:, i * N + nh * 512:
                                                i * N + (nh + 1) * 512],
                                start=(cc == 0), stop=(cc == 3),
                            )
                    yt = sb.tile([128, N], F32, tag="y", bufs=BUFS["y"])
                    nc.vector.tensor_add(yt[:], p_ps[:], xr_sb[(img, oc)][:])
                    nc.sync.dma_start(out=y_h[img, oc * 128:(oc + 1) * 128, :],
                                      in_=yt[:])

            # ---------- emission schedule (2 images, pipelined) ----------
            # warm the ACT exp table during the input DMAs
            warm = wp.tile([1, 1], F32, tag="warm", name="warm")
            nc.vector.memset(warm[:], 0.0)
            nc.scalar.activation(
                warm[:], warm[:], mybir.ActivationFunctionType.Exp)
            bq_sb = load_weights()
            alloc_on(0)

            # minimal preamble: first head (h=1) needs Q/K chunk 0 and the
            # first V pair; the rest drains as fillers inside the head loop
            emit_qkv(0, [0, 4])
            emit_v(0, [0, 1, 2, 3])

            head_order = list(range(8))  # pair (2k, 2k+1) completes at the
            # odd head; its transpose DMAs fire there

            fillers0 = [
                [lambda: emit_qkv(0, [1]),
                 lambda: (emit_qkv(0, [5]), emit_v(0, [4, 5])),
                 lambda: emit_v(0, [6, 7])],
                [lambda: emit_qkv(0, [2]), lambda: emit_qkv(0, [6])],
                [lambda: emit_qkv(0, [3]),
                 lambda: emit_qkv(0, [7]),
                 lambda: load_xm(1)],
                [lambda: emit_qkv(1, [0]), lambda: emit_qkv(1, [4])],
                [lambda: emit_qkv(1, [1]),
                 lambda: emit_qkv(1, [5]),
                 lambda: emit_v(1, [0, 1])],
                [lambda: emit_qkv(1, [2]),
                 lambda: emit_qkv(1, [6]),
                 lambda: emit_v(1, [2, 3])],
                [lambda: emit_qkv(1, [3]),
                 lambda: emit_qkv(1, [7]),
                 lambda: emit_v(1, [4, 5])],
                [lambda: emit_v(1, [6, 7]),
                 lambda: (load_xr(0), alloc_on(1))],
            ]
            for pos, h in enumerate(head_order):
                emit_head(0, h, 0, pos, filler=fillers0[pos])

            fillers1 = [
                None,
                [None, lambda: emit_proj(0, [0])],
                [None, lambda: emit_proj(0, [1])],
                [None, lambda: emit_proj(0, [2])],
                [None, lambda: emit_proj(0, [3])],
                [lambda: load_xr(1)],
                None,
                None,
            ]
            for pos, h in enumerate(head_order):
                emit_head(1, h, 1, pos, filler=fillers1[pos])
            emit_proj(1, range(4))

    _split_multi_waits(nc)
    return nc


_CACHE = {}


def _get_nc(mode=None):
    if "nc" not in _CACHE:
        _CACHE["nc"] = build_nc()
    return _CACHE["nc"]


def prepare_inputs(x, qkv_w, qkv_b, proj_w, proj_b):
    f8 = ml_dtypes.float8_e4m3
    x = np.asarray(x, np.float32).reshape(B, C, N)
    qkv_w = np.asarray(qkv_w, np.float32)
    qkv_b = np.asarray(qkv_b, np.float32)
    proj_w = np.asarray(proj_w, np.float32)
    proj_b = np.asarray(proj_b, np.float32)

    xm = np.ascontiguousarray(x.astype(f8))
    # residual with proj bias and the folded V-bias term (sum_m attn == 1)
    rbias = proj_b + proj_w.astype(ml_dtypes.bfloat16).astype(np.float32) @ qkv_b[2 * C:]
    xr = np.ascontiguousarray(x + rbias[None, :, None])
    wqkv = np.ascontiguousarray(qkv_w.T.astype(f8))
    pw = np.ascontiguousarray(proj_w.T.astype(ml_dtypes.bfloat16))
    bq = np.ascontiguousarray(qkv_b[:C].reshape(4, 128).T)

    in_maps = []
    for c in range(NCORES):
        sl = slice(c * BPC, (c + 1) * BPC)
        in_maps.append({
            "xm": xm[sl], "xr": xr[sl], "wqkv": wqkv, "pw": pw, "bq": bq,
        })
    return in_maps


def run(x, qkv_w, qkv_b, proj_w, proj_b, mode=None, **spmd_kwargs):
    nc = _get_nc()
    in_maps = prepare_inputs(x, qkv_w, qkv_b, proj_w, proj_b)
    res = run_bass_kernel_spmd(nc, in_maps, list(range(NCORES)), **spmd_kwargs)
    y = np.concatenate([np.asarray(res.results[c]["y"]) for c in range(NCORES)], axis=0)
    return res, y.reshape(B, C, 32, 32).astype(np.float32)


MM_MODE = "fp8dr"


def kernel(x, qkv_w, qkv_b, proj_w, proj_b):
    _, y = run(x, qkv_w, qkv_b, proj_w, proj_b)
    return y
